# revision 29
# baseline (speedup 1.0000x reference)
"""EnhancedAdaptiveLoRAPooling fused kernel for 8x Trainium2 NeuronCores.

Strategy v8: host-side routing + fp8 low-rank delta device kernel.

The reference output is y = x + delta(x) where delta is a rank-16 linear
map (current-task LoRA fused with the similarity-pooled LoRA).  All the
routing math (cosine/euclid sims, 4-layer MLP, top-3 + threshold,
weighted pooling, fusion weights) involves only KB-sized tensors, so it
runs on the host in f32 numpy and folds into two small matrices:
  Acomb [16, H] = [(1-fw)*S*A_cur ; fw*S*pooled_a]   (fp8, x32 scale)
  Bcomb [H, 16] = [B_cur | pooled_b]                 (bf16, scaled so
                                                      PSUM == int8 grid)
The device does only the O(B*S*H) work:
  in:  xT fp8 [H, TPC]     (1 byte/elem transport)
  v   = Acomb^T x          fp8 DoubleRow matmuls (2 k-tiles each)
  d   = Bcomb^T v          bf16 matmuls, K=16
  out: dT int8 [H, TPC]    (PSUM f32 -> int8 copies split DVE/ACT)
The host adds y = x + S_D * delta in f32 (x stays exact; only the tiny
delta carries fp8/int8 noise; measured end-to-end rel err ~7e-4).

Schedule notes (v8, from the v7 trace):
  - x tiles go on ONE ring in order so tile0 arrives at full aggregate
    DMA bandwidth instead of sharing with tile1.
  - dp bufs=3: with bufs=2 the delta matmul for chunk c+2 serialized
    behind the conversion of chunk c, adding ~1us per chunk pair.
  - v is computed in two 512-token groups at PSUM partitions 0/32
    (tile_position), so the v copy is [48, 512] (512 positions) instead
    of [16, 1024] (1024 positions) -- halves its engine cost.
  - SWDGE descriptor generation (~1us per DMA) executes ON the issuing
    engine, so stores go on the idle sync/gpsimd rings, never DVE/ACT.
  - Conversions alternate DVE (chunks 0,2,4) / ACT (1,3,5 + v copies).
"""

import numpy as np

B, S, H = 8, 4096, 768
N_TASKS, R = 16, 8
SCALING = 2.0
TOP_K = 3
NCORES = 8
TPC = (B * S) // NCORES          # tokens per core = 4096
TT = 2048                        # token tile (DMA granularity)
NTILE = TPC // TT                # 2
HT = 1024                        # compute half-tile
NCH = H // 128                   # 6 hidden chunks
GT = 512                         # v group token width (2 groups per half)

KA = 32.0                        # fp8 scale for Acomb
S_D = 0.5 / 127                  # int8 delta grid
_NBLOB = 48 + 384                # A2 fp8 (192B, M padded to 32) + Bcomb bf16

_PROGRAM = None


def _build_program():
    from contextlib import ExitStack

    import concourse.bass as bass  # noqa: F401
    import concourse.tile as tile
    from concourse import bacc, mybir

    f32 = mybir.dt.float32
    bf16 = mybir.dt.bfloat16
    fp8 = mybir.dt.float8e4
    i8 = mybir.dt.int8
    DR = mybir.MatmulPerfMode.DoubleRow

    nc = bacc.Bacc("TRN2", target_bir_lowering=False, debug=False)

    # x fp8 packed as f32 columns (4 fp8 per f32)
    xT = nc.dram_tensor("xT", [H, TPC // 4], f32, kind="ExternalInput").ap()
    wblob = nc.dram_tensor("wblob", [128, _NBLOB], f32,
                           kind="ExternalInput").ap()
    yT = nc.dram_tensor("yT", [H, TPC], i8, kind="ExternalOutput").ap()

    xT_r = xT.rearrange("(c p) t -> p c t", p=128)
    yT_r = yT.rearrange("(c p) t -> p c t", p=128)

    with tile.TileContext(nc) as tc:
        with ExitStack() as ctx:
            const = ctx.enter_context(tc.tile_pool(name="const", bufs=1))
            wblob_sb = const.tile([128, _NBLOB], f32, name="wblob_sb")
            nc.scalar.dma_start(out=wblob_sb, in_=wblob)
            # A2 [128, kp, i, 32] fp8 (DoubleRow stationary, 3 k-pairs;
            # stationary cols 16-31 zero). v_sb rows 32-127 are zeroed
            # once per buffer so the delta matmuls contract K=128 at the
            # full-array column rate (small-K runs at half rate).
            A2_sb = wblob_sb[:, 0:48].bitcast(fp8).rearrange(
                "p (k i m) -> p k i m", k=3, i=2)
            # Bc [128, c, 128] bf16; rows k<16 hold Bcomb rank k, rest zero
            Bc_sb = wblob_sb[:, 48:48 + 384].bitcast(bf16).rearrange(
                "p (c m) -> p c m", c=6)

            # x tiles: f32-typed DMA, fp8 view for compute; each tile is
            # split across the sync+gpsimd rings (chunks 0-2 / 3-5) so two
            # DGEs feed the DMA engines and the tile lands ~2x sooner.
            xp = ctx.enter_context(tc.tile_pool(name="xp", bufs=2))
            xts = []
            # gpsimd's descriptor generation lags sync by ~1us, so give it
            # the smaller first piece: queue FIFO then completes both
            # pieces of tile 0 at about the same time.
            splits = [4, 3]
            for it in range(NTILE):
                t0 = it * (TT // 4)
                sp = splits[it]
                xt = xp.tile([128, NCH, TT // 4], f32, tag="xt", name=f"xt{it}")
                nc.sync.dma_start(out=xt[:, 0:sp, :],
                                  in_=xT_r[:, 0:sp, t0:t0 + TT // 4])
                nc.gpsimd.dma_start(out=xt[:, sp:6, :],
                                    in_=xT_r[:, sp:6, t0:t0 + TT // 4])
                xts.append(xt.bitcast(fp8).rearrange(
                    "p c (g t) -> p c g t", g=1)[:, :, 0, :])  # [128, 6, TT]

            vp = ctx.enter_context(tc.tile_pool(name="vp", bufs=1, space="PSUM"))
            dp = ctx.enter_context(tc.tile_pool(name="dp", bufs=3, space="PSUM"))
            vsb = ctx.enter_context(tc.tile_pool(name="vsb", bufs=2))
            yp = ctx.enter_context(tc.tile_pool(name="yp", bufs=2))

            yts = [yp.tile([128, NCH, TT], i8, tag="yt", name=f"yt{it}")
                   for it in range(NTILE)]
            v_sbs = {}

            vcnt = [0]

            def emit_v(it, h):
                """v[32, HT] = Acomb^T x for half h of tile it.  v_sb rows
                32-127 are zeroed once per buffer (they multiply zero B
                rows; K=128 keeps the delta matmuls at full column rate)."""
                xt = xts[it]
                c0 = h * HT
                v_ps = vp.tile([32, HT], f32, tag="v", name="v_ps")
                v_sb = vsb.tile([128, HT], bf16, tag="v_sb", name=f"v{it}{h}")
                if vcnt[0] < 2:
                    for p0 in range(32, 128, 32):
                        nc.gpsimd.memset(v_sb[p0:p0 + 32, :], 0)
                for half in range(2):
                    for q in range(2 * half, 2 * half + 2):
                        o0 = c0 + q * 256
                        for kp in range(3):
                            nc.tensor.matmul(
                                v_ps[:, q * 256:(q + 1) * 256],
                                lhsT=A2_sb[:, kp, :, :],
                                rhs=xt[:, 2 * kp:2 * kp + 2, o0:o0 + 256],
                                start=(kp == 0), stop=(kp == 2),
                                perf_mode=DR)
                    # piecewise copy: piece 0 overlaps the second v block
                    s = slice(half * GT, (half + 1) * GT)
                    if (vcnt[0] + half) % 2 == 0:
                        nc.scalar.copy(v_sb[0:32, s], v_ps[:, s])
                    else:
                        nc.vector.tensor_scalar_mul(v_sb[0:32, s], v_ps[:, s], 1.0)
                vcnt[0] += 1
                v_sbs[(it, h)] = v_sb

            def emit_delta(it, h, store):
                """delta chunks for half h of tile it -> int8 yt + stores.

                store: "none" | "tile" (both halves, 2KiB desc) |
                       "half" (this half) | "chunks" (per chunk, low tail)
                """
                v_sb = v_sbs[(it, h)]
                yt = yts[it]
                c0 = h * HT
                t0 = it * TT + c0
                for c in range(NCH):
                    d_ps = dp.tile([128, HT], f32, tag="d", name="d_ps")
                    for g in range(2):
                        nc.tensor.matmul(
                            d_ps[:, g * GT:(g + 1) * GT],
                            lhsT=Bc_sb[:, c, :],
                            rhs=v_sb[:, g * GT:(g + 1) * GT],
                            start=True, stop=True)
                    dst = yt[:, c, c0:c0 + HT]
                    if c % 2 == 0:
                        nc.vector.tensor_scalar_mul(dst, d_ps, 1.0)
                    else:
                        nc.scalar.copy(dst, d_ps)
                    if store == "chunks" and c >= NCH - 2:
                        # final two parity stores right after their last conv
                        lo = c % 2
                        ring = nc.sync if lo == 0 else nc.gpsimd
                        ring.dma_start(out=yT_r[:, lo:NCH:2, t0:t0 + HT],
                                       in_=yt[:, lo:NCH:2, c0:c0 + HT])
                if store == "tile":
                    tt0 = it * TT
                    nc.gpsimd.dma_start(out=yT_r[:, 0:NCH:2, tt0:tt0 + TT],
                                        in_=yt[:, 0:NCH:2, :])
                    nc.sync.dma_start(out=yT_r[:, 1:NCH:2, tt0:tt0 + TT],
                                      in_=yt[:, 1:NCH:2, :])
                elif store == "half":
                    nc.gpsimd.dma_start(out=yT_r[:, 0:NCH:2, t0:t0 + HT],
                                        in_=yt[:, 0:NCH:2, c0:c0 + HT])
                    nc.sync.dma_start(out=yT_r[:, 1:NCH:2, t0:t0 + HT],
                                      in_=yt[:, 1:NCH:2, c0:c0 + HT])

            emit_v(0, 0)
            emit_delta(0, 0, "none")
            emit_v(0, 1)
            emit_delta(0, 1, "tile")
            emit_v(1, 0)
            emit_delta(1, 0, "half")
            emit_v(1, 1)
            emit_delta(1, 1, "chunks")

    nc.compile()
    return nc


def _get_program():
    global _PROGRAM
    if _PROGRAM is None:
        _PROGRAM = _build_program()
    return _PROGRAM


def _routing(inputs):
    """Host-side routing: returns Acomb [16,H] f32 (scaled), Bcomb [H,16]."""
    cur = np.asarray(inputs["task_embedding"], np.float32)
    la = np.asarray(inputs["loras_a"], np.float32)
    lb = np.asarray(inputs["loras_b"], np.float32)
    te = np.asarray(inputs["task_embeds"], np.float32)
    W1 = np.asarray(inputs["W1"], np.float32)
    W2 = np.asarray(inputs["W2"], np.float32)
    W3 = np.asarray(inputs["W3"], np.float32)
    W4 = np.asarray(inputs["W4"], np.float32)
    b1 = np.asarray(inputs["b1"], np.float32)
    b2 = np.asarray(inputs["b2"], np.float32)
    b3 = np.asarray(inputs["b3"], np.float32)
    b4 = np.asarray(inputs["b4"], np.float32)
    tid = int(np.asarray(inputs["current_task_id"]))

    cur_norm = np.linalg.norm(cur)
    emb_norms = np.linalg.norm(te, axis=-1)
    cos_sim = (te @ cur) / np.maximum(emb_norms * cur_norm, 1e-8)
    euclid = np.linalg.norm(te - cur[None, :], axis=-1)
    euclid_sim = 1.0 / (1.0 + euclid)
    comb = np.concatenate([np.broadcast_to(cur, te.shape), te], axis=-1)
    h = np.maximum(comb @ W1.T + b1, 0.0)
    h = np.maximum(h @ W2.T + b2, 0.0)
    h = np.maximum(h @ W3.T + b3, 0.0)
    nn_sim = 1.0 / (1.0 + np.exp(-(h @ W4.T + b4)))[..., 0]
    sims = 0.4 * cos_sim + 0.3 * euclid_sim + 0.3 * nn_sim

    top_idx = np.argpartition(-sims, TOP_K)[:TOP_K]
    top_vals = sims[top_idx]
    w = np.where(top_vals > 0.0, top_vals, 0.0)
    tw = float(w.sum())
    sw = tw if tw > 0 else 1.0
    pa = np.einsum('k,krh->rh', w, la[top_idx]) / sw
    pb = np.einsum('k,khr->hr', w, lb[top_idx]) / sw
    fw = min(cur_norm * 0.1, 0.5)
    c_cur = (1.0 - fw) * SCALING if tw > 0 else SCALING
    c_pool = fw * SCALING if tw > 0 else 0.0
    Acomb = np.concatenate([la[tid] * c_cur, pa * c_pool], axis=0)  # [16, H]
    Bcomb = np.concatenate([lb[tid], pb], axis=1)                   # [H, 16]
    return Acomb, Bcomb


def _make_in_maps(inputs):
    import ml_dtypes
    bf16 = ml_dtypes.bfloat16
    fp8 = ml_dtypes.float8_e4m3

    hs = np.asarray(inputs["hidden_states"], np.float32)
    Acomb, Bcomb = _routing(inputs)

    # A2[p, kp, i, m] = (KA*Acomb)[m, (2kp+i)*128 + p] for m<16, 0 pad to 32
    As = np.zeros((32, 768), np.float32)
    As[0:16] = Acomb * KA
    A2 = np.ascontiguousarray(
        As.astype(fp8).reshape(32, 3, 2, 128).transpose(3, 1, 2, 0))
    # Bpack[k, c, m] = Bdev[c*128+m, k], bf16, rows k>=16 zero
    Bdev = (Bcomb / (KA * S_D)).astype(bf16)          # [768, 16]
    Bpack = np.ascontiguousarray(
        Bdev.reshape(6, 128, 16).transpose(2, 0, 1))  # [16, 6, 128]

    wblob = np.zeros((128, _NBLOB), np.float32)
    wblob[:, 0:48] = A2.reshape(128, 192).view(np.float32)
    wblob[0:16, 48:48 + 384] = Bpack.reshape(16, 768).view(np.float32)

    x2 = hs.reshape(B * S, H)
    in_maps = []
    for i in range(NCORES):
        shard = np.ascontiguousarray(
            x2[i * TPC:(i + 1) * TPC].T).astype(fp8)  # [H, TPC]
        in_maps.append({"xT": shard.view(np.float32), "wblob": wblob})
    return in_maps


def kernel(**inputs):
    from concourse.bass_utils import run_bass_kernel_spmd

    nc = _get_program()
    in_maps = _make_in_maps(inputs)
    res = run_bass_kernel_spmd(nc, in_maps, core_ids=list(range(NCORES)))
    hs = np.asarray(inputs["hidden_states"], np.float32)
    out = np.empty((B * S, H), np.float32)
    x2 = hs.reshape(B * S, H)
    for i, r in enumerate(res.results):
        d = r["yT"].T.astype(np.float32)
        out[i * TPC:(i + 1) * TPC] = x2[i * TPC:(i + 1) * TPC] + d * S_D
    return out.reshape(B, S, H)


# revision 32
# speedup vs baseline: 1.0408x; 1.0408x over previous
"""EnhancedAdaptiveLoRAPooling fused kernel for 8x Trainium2 NeuronCores.

Strategy v8: host-side routing + fp8 low-rank delta device kernel.

The reference output is y = x + delta(x) where delta is a rank-16 linear
map (current-task LoRA fused with the similarity-pooled LoRA).  All the
routing math (cosine/euclid sims, 4-layer MLP, top-3 + threshold,
weighted pooling, fusion weights) involves only KB-sized tensors, so it
runs on the host in f32 numpy and folds into two small matrices:
  Acomb [16, H] = [(1-fw)*S*A_cur ; fw*S*pooled_a]   (fp8, x32 scale)
  Bcomb [H, 16] = [B_cur | pooled_b]                 (bf16, scaled so
                                                      PSUM == int8 grid)
The device does only the O(B*S*H) work:
  in:  xT fp8 [H, TPC]     (1 byte/elem transport)
  v   = Acomb^T x          fp8 DoubleRow matmuls (2 k-tiles each)
  d   = Bcomb^T v          bf16 matmuls, K=16
  out: dT int8 [H, TPC]    (PSUM f32 -> int8 copies split DVE/ACT)
The host adds y = x + S_D * delta in f32 (x stays exact; only the tiny
delta carries fp8/int8 noise; measured end-to-end rel err ~7e-4).

Schedule notes (v8, from the v7 trace):
  - x tiles go on ONE ring in order so tile0 arrives at full aggregate
    DMA bandwidth instead of sharing with tile1.
  - dp bufs=3: with bufs=2 the delta matmul for chunk c+2 serialized
    behind the conversion of chunk c, adding ~1us per chunk pair.
  - v is computed in two 512-token groups at PSUM partitions 0/32
    (tile_position), so the v copy is [48, 512] (512 positions) instead
    of [16, 1024] (1024 positions) -- halves its engine cost.
  - SWDGE descriptor generation (~1us per DMA) executes ON the issuing
    engine, so stores go on the idle sync/gpsimd rings, never DVE/ACT.
  - Conversions alternate DVE (chunks 0,2,4) / ACT (1,3,5 + v copies).
"""

import numpy as np

B, S, H = 8, 4096, 768
N_TASKS, R = 16, 8
SCALING = 2.0
TOP_K = 3
NCORES = 8
TPC = (B * S) // NCORES          # tokens per core = 4096
TT = 2048                        # token tile (DMA granularity)
NTILE = TPC // TT                # 2
HT = 1024                        # compute half-tile
NCH = H // 128                   # 6 hidden chunks
GT = 512                         # v group token width (2 groups per half)

KA = 32.0                        # fp8 scale for Acomb
S_D = 0.5 / 127                  # int8 delta grid
_NBLOB = 48 + 384                # A2 fp8 (192B, M padded to 32) + Bcomb bf16

_PROGRAM = None


def _build_program():
    from contextlib import ExitStack

    import concourse.bass as bass  # noqa: F401
    import concourse.tile as tile
    from concourse import bacc, mybir

    f32 = mybir.dt.float32
    bf16 = mybir.dt.bfloat16
    fp8 = mybir.dt.float8e4
    i8 = mybir.dt.int8
    DR = mybir.MatmulPerfMode.DoubleRow

    nc = bacc.Bacc("TRN2", target_bir_lowering=False, debug=False)

    # x fp8 packed as f32 columns (4 fp8 per f32)
    xT = nc.dram_tensor("xT", [H, TPC // 4], f32, kind="ExternalInput").ap()
    wblob = nc.dram_tensor("wblob", [128, _NBLOB], f32,
                           kind="ExternalInput").ap()
    yT = nc.dram_tensor("yT", [H, TPC], i8, kind="ExternalOutput").ap()

    xT_r = xT.rearrange("(c p) t -> p c t", p=128)
    yT_r = yT.rearrange("(c p) t -> p c t", p=128)

    with tile.TileContext(nc) as tc:
        with ExitStack() as ctx:
            const = ctx.enter_context(tc.tile_pool(name="const", bufs=1))
            wblob_sb = const.tile([128, _NBLOB], f32, name="wblob_sb")
            nc.scalar.dma_start(out=wblob_sb, in_=wblob)
            # A2 [128, kp, i, 32] fp8 (DoubleRow stationary, 3 k-pairs;
            # stationary cols 16-31 zero). v_sb rows 32-127 are zeroed
            # once per buffer so the delta matmuls contract K=128 at the
            # full-array column rate (small-K runs at half rate).
            A2_sb = wblob_sb[:, 0:48].bitcast(fp8).rearrange(
                "p (k i m) -> p k i m", k=3, i=2)
            # Bc [128, c, 128] bf16; rows k<16 hold Bcomb rank k, rest zero
            Bc_sb = wblob_sb[:, 48:48 + 384].bitcast(bf16).rearrange(
                "p (c m) -> p c m", c=6)

            # x tiles: f32-typed DMA, fp8 view for compute; each tile is
            # split across the sync+gpsimd rings (chunks 0-2 / 3-5) so two
            # DGEs feed the DMA engines and the tile lands ~2x sooner.
            xp = ctx.enter_context(tc.tile_pool(name="xp", bufs=2))
            xts = []
            # gpsimd's descriptor generation lags sync by ~1us, so give it
            # the smaller first piece: queue FIFO then completes both
            # pieces of tile 0 at about the same time.
            splits = [4, 3]
            for it in range(NTILE):
                t0 = it * (TT // 4)
                sp = splits[it]
                xt = xp.tile([128, NCH, TT // 4], f32, tag="xt", name=f"xt{it}")
                nc.sync.dma_start(out=xt[:, 0:sp, :],
                                  in_=xT_r[:, 0:sp, t0:t0 + TT // 4])
                nc.gpsimd.dma_start(out=xt[:, sp:6, :],
                                    in_=xT_r[:, sp:6, t0:t0 + TT // 4])
                xts.append(xt.bitcast(fp8).rearrange(
                    "p c (g t) -> p c g t", g=1)[:, :, 0, :])  # [128, 6, TT]

            vp = ctx.enter_context(tc.tile_pool(name="vp", bufs=1, space="PSUM"))
            dp = ctx.enter_context(tc.tile_pool(name="dp", bufs=3, space="PSUM"))
            yp = ctx.enter_context(tc.tile_pool(name="yp", bufs=2))

            yts = [yp.tile([128, NCH, TT], i8, tag="yt", name=f"yt{it}")
                   for it in range(NTILE)]
            v_sbs = {}

            # persistent v double-buffer; rows 32-127 zeroed once (they
            # multiply zero B rows; K=128 keeps delta at full column rate)
            vpers = [const.tile([128, HT], bf16, name=f"vbuf{j}")
                     for j in range(2)]
            for j in range(2):
                for p0 in range(32, 128, 32):
                    nc.gpsimd.memset(vpers[j][p0:p0 + 32, :], 0)

            # PE warmup: ramp the clock while waiting for x0 (reads wblob,
            # writes scratch psum; results unused)
            warm_rhs = wblob_sb[:, 48:304].bitcast(bf16)
            for i in range(14):
                w_ps = dp.tile([128, 512], f32, tag="d", name="warm")
                nc.tensor.matmul(w_ps, lhsT=Bc_sb[:, 0, :], rhs=warm_rhs,
                                 start=True, stop=True)

            vcnt = [0]

            def emit_v(it, h):
                """v[32, HT] = Acomb^T x for half h of tile it.  v_sb rows
                32-127 are zeroed once per buffer (they multiply zero B
                rows; K=128 keeps the delta matmuls at full column rate)."""
                xt = xts[it]
                c0 = h * HT
                v_ps = vp.tile([32, HT], f32, tag="v", name="v_ps")
                v_sb = vpers[(2 * it + h) % 2]
                for half in range(2):
                    for q in range(2 * half, 2 * half + 2):
                        o0 = c0 + q * 256
                        for kp in range(3):
                            nc.tensor.matmul(
                                v_ps[:, q * 256:(q + 1) * 256],
                                lhsT=A2_sb[:, kp, :, :],
                                rhs=xt[:, 2 * kp:2 * kp + 2, o0:o0 + 256],
                                start=(kp == 0), stop=(kp == 2),
                                perf_mode=DR)
                    # piecewise copy: piece 0 overlaps the second v block
                    s = slice(half * GT, (half + 1) * GT)
                    if (vcnt[0] + half) % 2 == 0:
                        nc.scalar.copy(v_sb[0:32, s], v_ps[:, s])
                    else:
                        nc.vector.tensor_scalar_mul(v_sb[0:32, s], v_ps[:, s], 1.0)
                vcnt[0] += 1
                v_sbs[(it, h)] = v_sb

            def emit_delta(it, h, store):
                """delta chunks for half h of tile it -> int8 yt + stores.

                store: "none" | "tile" (both halves, 2KiB desc) |
                       "half" (this half) | "chunks" (per chunk, low tail)
                """
                v_sb = v_sbs[(it, h)]
                yt = yts[it]
                c0 = h * HT
                t0 = it * TT + c0
                for c in range(NCH):
                    d_ps = dp.tile([128, HT], f32, tag="d", name="d_ps")
                    for g in range(2):
                        nc.tensor.matmul(
                            d_ps[:, g * GT:(g + 1) * GT],
                            lhsT=Bc_sb[:, c, :],
                            rhs=v_sb[:, g * GT:(g + 1) * GT],
                            start=True, stop=True)
                    dst = yt[:, c, c0:c0 + HT]
                    if c % 2 == 0:
                        nc.vector.tensor_scalar_mul(dst, d_ps, 1.0)
                    else:
                        nc.scalar.copy(dst, d_ps)
                    if store == "chunks" and c >= NCH - 2:
                        # final two parity stores right after their last conv
                        lo = c % 2
                        ring = nc.sync if lo == 0 else nc.gpsimd
                        ring.dma_start(out=yT_r[:, lo:NCH:2, t0:t0 + HT],
                                       in_=yt[:, lo:NCH:2, c0:c0 + HT])
                if store == "tile":
                    tt0 = it * TT
                    nc.gpsimd.dma_start(out=yT_r[:, 0:NCH:2, tt0:tt0 + TT],
                                        in_=yt[:, 0:NCH:2, :])
                    nc.sync.dma_start(out=yT_r[:, 1:NCH:2, tt0:tt0 + TT],
                                      in_=yt[:, 1:NCH:2, :])
                elif store == "half":
                    nc.gpsimd.dma_start(out=yT_r[:, 0:NCH:2, t0:t0 + HT],
                                        in_=yt[:, 0:NCH:2, c0:c0 + HT])
                    nc.sync.dma_start(out=yT_r[:, 1:NCH:2, t0:t0 + HT],
                                      in_=yt[:, 1:NCH:2, c0:c0 + HT])

            emit_v(0, 0)
            emit_delta(0, 0, "none")
            emit_v(0, 1)
            emit_delta(0, 1, "tile")
            emit_v(1, 0)
            emit_delta(1, 0, "half")
            emit_v(1, 1)
            emit_delta(1, 1, "chunks")

    nc.compile()
    return nc


def _get_program():
    global _PROGRAM
    if _PROGRAM is None:
        _PROGRAM = _build_program()
    return _PROGRAM


def _routing(inputs):
    """Host-side routing: returns Acomb [16,H] f32 (scaled), Bcomb [H,16]."""
    cur = np.asarray(inputs["task_embedding"], np.float32)
    la = np.asarray(inputs["loras_a"], np.float32)
    lb = np.asarray(inputs["loras_b"], np.float32)
    te = np.asarray(inputs["task_embeds"], np.float32)
    W1 = np.asarray(inputs["W1"], np.float32)
    W2 = np.asarray(inputs["W2"], np.float32)
    W3 = np.asarray(inputs["W3"], np.float32)
    W4 = np.asarray(inputs["W4"], np.float32)
    b1 = np.asarray(inputs["b1"], np.float32)
    b2 = np.asarray(inputs["b2"], np.float32)
    b3 = np.asarray(inputs["b3"], np.float32)
    b4 = np.asarray(inputs["b4"], np.float32)
    tid = int(np.asarray(inputs["current_task_id"]))

    cur_norm = np.linalg.norm(cur)
    emb_norms = np.linalg.norm(te, axis=-1)
    cos_sim = (te @ cur) / np.maximum(emb_norms * cur_norm, 1e-8)
    euclid = np.linalg.norm(te - cur[None, :], axis=-1)
    euclid_sim = 1.0 / (1.0 + euclid)
    comb = np.concatenate([np.broadcast_to(cur, te.shape), te], axis=-1)
    h = np.maximum(comb @ W1.T + b1, 0.0)
    h = np.maximum(h @ W2.T + b2, 0.0)
    h = np.maximum(h @ W3.T + b3, 0.0)
    nn_sim = 1.0 / (1.0 + np.exp(-(h @ W4.T + b4)))[..., 0]
    sims = 0.4 * cos_sim + 0.3 * euclid_sim + 0.3 * nn_sim

    top_idx = np.argpartition(-sims, TOP_K)[:TOP_K]
    top_vals = sims[top_idx]
    w = np.where(top_vals > 0.0, top_vals, 0.0)
    tw = float(w.sum())
    sw = tw if tw > 0 else 1.0
    pa = np.einsum('k,krh->rh', w, la[top_idx]) / sw
    pb = np.einsum('k,khr->hr', w, lb[top_idx]) / sw
    fw = min(cur_norm * 0.1, 0.5)
    c_cur = (1.0 - fw) * SCALING if tw > 0 else SCALING
    c_pool = fw * SCALING if tw > 0 else 0.0
    Acomb = np.concatenate([la[tid] * c_cur, pa * c_pool], axis=0)  # [16, H]
    Bcomb = np.concatenate([lb[tid], pb], axis=1)                   # [H, 16]
    return Acomb, Bcomb


def _make_in_maps(inputs):
    import ml_dtypes
    bf16 = ml_dtypes.bfloat16
    fp8 = ml_dtypes.float8_e4m3

    hs = np.asarray(inputs["hidden_states"], np.float32)
    Acomb, Bcomb = _routing(inputs)

    # A2[p, kp, i, m] = (KA*Acomb)[m, (2kp+i)*128 + p] for m<16, 0 pad to 32
    As = np.zeros((32, 768), np.float32)
    As[0:16] = Acomb * KA
    A2 = np.ascontiguousarray(
        As.astype(fp8).reshape(32, 3, 2, 128).transpose(3, 1, 2, 0))
    # Bpack[k, c, m] = Bdev[c*128+m, k], bf16, rows k>=16 zero
    Bdev = (Bcomb / (KA * S_D)).astype(bf16)          # [768, 16]
    Bpack = np.ascontiguousarray(
        Bdev.reshape(6, 128, 16).transpose(2, 0, 1))  # [16, 6, 128]

    wblob = np.zeros((128, _NBLOB), np.float32)
    wblob[:, 0:48] = A2.reshape(128, 192).view(np.float32)
    wblob[0:16, 48:48 + 384] = Bpack.reshape(16, 768).view(np.float32)

    x2 = hs.reshape(B * S, H)
    in_maps = []
    for i in range(NCORES):
        shard = np.ascontiguousarray(
            x2[i * TPC:(i + 1) * TPC].T).astype(fp8)  # [H, TPC]
        in_maps.append({"xT": shard.view(np.float32), "wblob": wblob})
    return in_maps


def kernel(**inputs):
    from concourse.bass_utils import run_bass_kernel_spmd

    nc = _get_program()
    in_maps = _make_in_maps(inputs)
    res = run_bass_kernel_spmd(nc, in_maps, core_ids=list(range(NCORES)))
    hs = np.asarray(inputs["hidden_states"], np.float32)
    out = np.empty((B * S, H), np.float32)
    x2 = hs.reshape(B * S, H)
    for i, r in enumerate(res.results):
        d = r["yT"].T.astype(np.float32)
        out[i * TPC:(i + 1) * TPC] = x2[i * TPC:(i + 1) * TPC] + d * S_D
    return out.reshape(B, S, H)
